# revision 9
# baseline (speedup 1.0000x reference)
"""Trainium2 Bass kernel for nn_Decomposeable (decomposable attention).

Sharding: data-parallel over batch — 8 cores x 16 examples.
Layout strategy per core (T = 4096 tokens per side, blocks of 128 tokens,
token t = block*128 + p):
  - gather emb rows (bf16 cast in DMA), rowwise rsqrt-norm via exp(-0.5*ln(ss))
  - PE-transpose to feature-major eT [300, T]; project to x^T [200, T] (bf16)
  - PE-transpose x^T -> token-major x_tok blocks (attention values)
  - F-MLP feature-major; sim + simT per example on PE; ACT exp with
    per-partition mask scale + accumulated row sums (softmax without
    normalizing the matrix: fold 1/Z into the attention-output evacuation)
  - G layer1 feature-major (concat via K-chunks), layer2 token-major with
    bias via ones-row; relu+mask+evac fused; masked sum via indicator matmul
  - H-MLP + output head on [16, 400] (bias via ones-rows)
All matmul inputs bf16, fp32 PSUM accumulation.
"""
import sys
import numpy as np

for _p in ('/opt/trn_rl_repo', '/root/.axon_site'):
    if _p not in sys.path:
        sys.path.insert(0, _p)

import ml_dtypes

bfloat16 = ml_dtypes.bfloat16

B, S, V, E, D, C = 128, 256, 50000, 300, 200, 3
NCORES = 8
BPC = B // NCORES          # 16 examples per core
T = BPC * S                # 4096 tokens per side per core
NB = T // 128              # 32 blocks
NGRP = NB // 4             # 8 groups (512 tokens)

_cache = {}


def _build(debug_taps=False):
    from concourse import bass, bacc, mybir, tile

    F32 = mybir.dt.float32
    BF16 = mybir.dt.bfloat16
    I32 = mybir.dt.int32
    AF = mybir.ActivationFunctionType
    OP = mybir.AluOpType
    X = mybir.AxisListType.X

    nc = bacc.Bacc(None, num_swdge_queues=4)

    # ---- dram I/O ----
    emb = nc.dram_tensor("emb", [V, E], F32, kind="ExternalInput")
    xi = [nc.dram_tensor(f"x{s}i", [128, NB], I32, kind="ExternalInput") for s in (1, 2)]
    thr = [nc.dram_tensor(f"thr{s}", [NB], F32, kind="ExternalInput") for s in (1, 2)]
    iota_in = nc.dram_tensor("iota", [128, 1], F32, kind="ExternalInput")
    identb_in = nc.dram_tensor("identb", [128, 128], BF16, kind="ExternalInput")
    qsel_in = nc.dram_tensor("qsel", [128, 2 * BPC - 1], BF16, kind="ExternalInput")
    wproj_in = nc.dram_tensor("wproj", [E, D], BF16, kind="ExternalInput")
    fw1_in = nc.dram_tensor("fw1", [D, D], BF16, kind="ExternalInput")
    fw2_in = nc.dram_tensor("fw2", [D, D], BF16, kind="ExternalInput")
    gw1_in = nc.dram_tensor("gw1", [2 * D, D], BF16, kind="ExternalInput")
    gw2e_in = nc.dram_tensor("gw2e", [D + 1, D], BF16, kind="ExternalInput")
    hw1e_in = nc.dram_tensor("hw1e", [2 * D + 1, D], BF16, kind="ExternalInput")
    hw2e_in = nc.dram_tensor("hw2e", [D + 1, D], BF16, kind="ExternalInput")
    woute_in = nc.dram_tensor("woute", [D + 1, C], BF16, kind="ExternalInput")
    fb1_in = nc.dram_tensor("fb1", [D, 1], F32, kind="ExternalInput")
    fb2_in = nc.dram_tensor("fb2", [D, 1], F32, kind="ExternalInput")
    gb1_in = nc.dram_tensor("gb1", [D, 1], F32, kind="ExternalInput")
    y_out = nc.dram_tensor("y", [C, BPC], F32, kind="ExternalOutput")

    taps = {}

    def tap(name, shape, dtype=F32):
        if debug_taps:
            taps[name] = nc.dram_tensor(f"tap_{name}", shape, dtype, kind="ExternalOutput")
            return taps[name]
        return None

    from concourse.tile import TileContext, add_dep_helper

    from contextlib import ExitStack
    with TileContext(nc) as tc, ExitStack() as stk:
        wp = stk.enter_context(tc.tile_pool(name="wp", bufs=1))
        sp = stk.enter_context(tc.tile_pool(name="sp", bufs=1))
        ep = stk.enter_context(tc.tile_pool(name="ep", bufs=8))
        etp = stk.enter_context(tc.tile_pool(name="etp", bufs=2))
        xp = stk.enter_context(tc.tile_pool(name="xp", bufs=2))
        fp = stk.enter_context(tc.tile_pool(name="fp", bufs=2))
        ap_ = stk.enter_context(tc.tile_pool(name="ap", bufs=2))
        gp = stk.enter_context(tc.tile_pool(name="gp", bufs=3))
        vp = stk.enter_context(tc.tile_pool(name="vp", bufs=1))
        trp = stk.enter_context(tc.tile_pool(name="trp", bufs=2, space="PSUM"))
        mmp = stk.enter_context(tc.tile_pool(name="mmp", bufs=3, space="PSUM"))
        vpp = stk.enter_context(tc.tile_pool(name="vpp", bufs=1, space="PSUM"))

        # ---------- setup ----------
        idx_t = [wp.tile([128, NB], I32, tag=f"idx{s}", name=f"idx{s}") for s in range(2)]
        for s in range(2):
            nc.sync.dma_start(out=idx_t[s][:], in_=xi[s][:])
        iota_t = wp.tile([128, 1], F32, tag="iota", name="iota")
        nc.sync.dma_start(out=iota_t[:], in_=iota_in[:])
        identb = wp.tile([128, 128], BF16, tag="identb", name="identb")
        nc.sync.dma_start(out=identb[:], in_=identb_in[:])
        qsel = wp.tile([128, 2 * BPC - 1], BF16, tag="qsel", name="qsel")
        nc.sync.dma_start(out=qsel[:], in_=qsel_in[:])

        # thresholds replicated across partitions, then mask = (iota < thr)
        mask_tok = []
        for s in range(2):
            thr_rep = sp.tile([128, NB], F32, tag=f"thrrep{s}", name=f"thrrep{s}")
            nc.gpsimd.dma_start(
                out=thr_rep[:],
                in_=bass.AP(tensor=thr[s], offset=0, ap=[[0, 128], [1, NB]]))
            m = wp.tile([128, NB], F32, tag=f"mask{s}", name=f"mask{s}")
            nc.vector.tensor_tensor(
                out=m[:], in0=iota_t[:].to_broadcast([128, NB]), in1=thr_rep[:],
                op=OP.is_lt)
            mask_tok.append(m)

        # weights (bf16, chunked by K)
        def load_rows(dram, r0, r1, cols, tag):
            t = wp.tile([r1 - r0, cols], BF16, tag=tag, name=tag)
            nc.sync.dma_start(out=t[:], in_=dram[r0:r1, :])
            return t

        wproj_k = [load_rows(wproj_in, k * 128, min((k + 1) * 128, E), D, f"wp{k}")
                   for k in range(3)]
        fw1_k = [load_rows(fw1_in, 0, 128, D, "fw1a"), load_rows(fw1_in, 128, D, D, "fw1b")]
        fw2_k = [load_rows(fw2_in, 0, 128, D, "fw2a"), load_rows(fw2_in, 128, D, D, "fw2b")]
        gw1_k = [load_rows(gw1_in, 0, 128, D, "gw1a"), load_rows(gw1_in, 128, 200, D, "gw1b"),
                 load_rows(gw1_in, 200, 328, D, "gw1c"), load_rows(gw1_in, 328, 400, D, "gw1d")]
        gw2_k = [load_rows(gw2e_in, 0, 128, D, "gw2a"), load_rows(gw2e_in, 128, 201, D, "gw2b")]
        hw1_k = [load_rows(hw1e_in, 0, 128, D, "hw1a"), load_rows(hw1e_in, 128, 200, D, "hw1b"),
                 load_rows(hw1e_in, 200, 328, D, "hw1c"), load_rows(hw1e_in, 328, 401, D, "hw1d")]
        hw2_k = [load_rows(hw2e_in, 0, 128, D, "hw2a"), load_rows(hw2e_in, 128, 201, D, "hw2b")]
        wout_k = [load_rows(woute_in, 0, 128, C, "woa"), load_rows(woute_in, 128, 201, C, "wob")]

        fb1_t = [wp.tile([128, 1], F32, tag="fb1a", name="fb1a"), wp.tile([72, 1], F32, tag="fb1b", name="fb1b")]
        fb2_t = [wp.tile([128, 1], F32, tag="fb2a", name="fb2a"), wp.tile([72, 1], F32, tag="fb2b", name="fb2b")]
        gb1_t = [wp.tile([128, 1], F32, tag="gb1a", name="gb1a"), wp.tile([72, 1], F32, tag="gb1b", name="gb1b")]
        for (dr, ts_) in ((fb1_in, fb1_t), (fb2_in, fb2_t), (gb1_in, gb1_t)):
            nc.sync.dma_start(out=ts_[0][:], in_=dr[0:128, :])
            nc.sync.dma_start(out=ts_[1][:], in_=dr[128:200, :])

        # per-side norm state
        ss_t = [sp.tile([128, NB], F32, tag=f"ss{s}", name=f"ss{s}") for s in range(2)]
        rs_t = [sp.tile([128, NB], F32, tag=f"rs{s}", name=f"rs{s}") for s in range(2)]
        ln_t = [sp.tile([128, NB], F32, tag=f"ln{s}", name=f"ln{s}") for s in range(2)]

        # v accumulation psums (live whole kernel)
        v_ps = [vpp.tile([BPC, D], F32, tag=f"vps{s}", name=f"vps{s}") for s in range(2)]

        MCH = ((0, 128), (128, 200))  # feature M/K chunks of D=200

        gather_n = 0
        e_hist = []  # (tile, last_consumer_instruction) per gather, for WAR deps
        EBUFS = 8

        def gather_block(s, c):
            nonlocal gather_n
            e = ep.tile([128, E], BF16, tag="e", name="e")
            g = nc.gpsimd.indirect_dma_start(
                out=e[:], out_offset=None, in_=emb[:],
                in_offset=bass.IndirectOffsetOnAxis(ap=idx_t[s][:, c:c + 1], axis=0))
            qn = gather_n % 4
            if qn:
                g.ins.queue = f"qPoolDynamic{qn}"
            if len(e_hist) >= EBUFS:
                prev = e_hist[len(e_hist) - EBUFS][1]
                if prev is not None:
                    add_dep_helper(g.ins, prev.ins, True, "gather WAR on recycled e slot")
            e_hist.append([e, None])
            gather_n += 1
            return e, len(e_hist) - 1

        # ---------- main loop ----------
        f_T = {}   # (s) -> (fa, fb) group tiles, recreated per group
        for g in range(NGRP):
            c0 = g * 4
            xtok = {}
            xpT = {}
            for s in range(2):
                # gather + sumsq for 4 blocks
                eb = []
                for c in range(c0, c0 + 4):
                    e, hidx = gather_block(s, c)
                    sq = ep.tile([128, E], F32, tag="sq", name="sq")
                    st = nc.vector.scalar_tensor_tensor(
                        out=sq[:], in0=e[:], scalar=1.0, in1=e[:],
                        op0=OP.mult, op1=OP.mult, accum_out=ss_t[s][:, c:c + 1])
                    eb.append((e, hidx, c))
                # rsqrt wave: rs = exp(-0.5 * ln(ss))
                nc.scalar.activation(out=ln_t[s][:, c0:c0 + 4], in_=ss_t[s][:, c0:c0 + 4],
                                     func=AF.Ln)
                nc.scalar.activation(out=rs_t[s][:, c0:c0 + 4], in_=ln_t[s][:, c0:c0 + 4],
                                     func=AF.Exp, scale=-0.5)
                # scale + transpose to eT
                eT = [etp.tile([128, 512], BF16, tag=f"eT{s}{k}", name=f"eT{s}{k}") for k in range(3)]
                for wv in range(2):  # waves of 2 blocks
                    tr = trp.tile([128, 3, 256], BF16, tag="tr", name="tr")
                    for half in range(2):
                        e, hidx, c = eb[wv * 2 + half]
                        ebf = ep.tile([128, E], BF16, tag="ebf", name="ebf")
                        sc = nc.vector.tensor_scalar(
                            out=ebf[:], in0=e[:], scalar1=rs_t[s][:, c:c + 1],
                            scalar2=None, op0=OP.mult)
                        e_hist[hidx][1] = sc
                        for k in range(3):
                            ksz = 128 if k < 2 else E - 256
                            nc.tensor.transpose(
                                out=tr[:ksz, k, half * 128:(half + 1) * 128],
                                in_=ebf[:, k * 128:k * 128 + ksz],
                                identity=identb[:])
                    for k in range(3):
                        ksz = 128 if k < 2 else E - 256
                        nc.vector.tensor_copy(
                            out=eT[k][:ksz, wv * 256:(wv + 1) * 256],
                            in_=tr[:ksz, k, :])
                # projection -> xpT (feature-major x^T, bf16)
                xpT[s] = [xp.tile([128, 512], BF16, tag=f"xpT{s}0", name=f"xpTa{s}"),
                          xp.tile([72, 512], BF16, tag=f"xpT{s}1", name=f"xpTb{s}")]
                for mi, (m0, m1) in enumerate(MCH):
                    ps = mmp.tile([128, 512], F32, tag="mm", name="mm")
                    for k in range(3):
                        ksz = 128 if k < 2 else E - 256
                        nc.tensor.matmul(
                            out=ps[:m1 - m0, :], lhsT=wproj_k[k][:ksz, m0:m1],
                            rhs=eT[k][:ksz, :], start=(k == 0), stop=(k == 2))
                    nc.scalar.copy(out=xpT[s][mi][:, :], in_=ps[:m1 - m0, :])
                # token-major x blocks (for attention values)
                xtok[s] = xp.tile([128, 4, D], BF16, tag=f"xtok{s}", name=f"xtok{s}")
                for blk in range(4):
                    tr = trp.tile([128, 3, 256], BF16, tag="tr", name="tr")
                    for mi, (m0, m1) in enumerate(MCH):
                        nc.tensor.transpose(
                            out=tr[:128, 0, m0:m1],
                            in_=xpT[s][mi][:m1 - m0, blk * 128:(blk + 1) * 128],
                            identity=identb[:m1 - m0, :m1 - m0])
                    nc.vector.tensor_copy(out=xtok[s][:, blk, :], in_=tr[:, 0, 0:D])
                # F MLP (feature-major)
                fh = [fp.tile([128, 512], BF16, tag=f"fh{s}0", name=f"fha{s}"),
                      fp.tile([72, 512], BF16, tag=f"fh{s}1", name=f"fhb{s}")]
                for mi, (m0, m1) in enumerate(MCH):
                    ps = mmp.tile([128, 512], F32, tag="mm", name="mm")
                    for ki, (k0, k1) in enumerate(MCH):
                        nc.tensor.matmul(
                            out=ps[:m1 - m0, :], lhsT=fw1_k[ki][:k1 - k0, m0:m1],
                            rhs=xpT[s][ki][:k1 - k0, :], start=(ki == 0), stop=(ki == 1))
                    nc.scalar.activation(out=fh[mi][:, :], in_=ps[:m1 - m0, :],
                                         func=AF.Relu, bias=fb1_t[mi][:])
                fT = [fp.tile([128, 512], BF16, tag=f"fT{s}0", name=f"fTa{s}"),
                      fp.tile([72, 512], BF16, tag=f"fT{s}1", name=f"fTb{s}")]
                for mi, (m0, m1) in enumerate(MCH):
                    ps = mmp.tile([128, 512], F32, tag="mm", name="mm")
                    for ki, (k0, k1) in enumerate(MCH):
                        nc.tensor.matmul(
                            out=ps[:m1 - m0, :], lhsT=fw2_k[ki][:k1 - k0, m0:m1],
                            rhs=fh[ki][:k1 - k0, :], start=(ki == 0), stop=(ki == 1))
                    nc.scalar.activation(out=fT[mi][:, :], in_=ps[:m1 - m0, :],
                                         func=AF.Relu, bias=fb2_t[mi][:])
                f_T[s] = fT

                if debug_taps and g == 0 and s == 0:
                    for nm, src in (("eT0", eT[0]), ("xpT0", xpT[0][0]),
                                    ("fT0", fT[0])):
                        t = tap(nm, [128, 512], BF16)
                        nc.sync.dma_start(out=t[:], in_=src[:])
                    t = tap("xtok0", [128, 4 * D], BF16)
                    nc.sync.dma_start(out=t[:], in_=xtok[0][:].rearrange("p a b -> p (a b)"))

            # ---- attention + G for the 2 examples of this group ----
            # group-level betaT/alphaT (feature-major attention outputs)
            attT = {0: [ap_.tile([128, 512], BF16, tag="betaTa", name="betaTa"),
                        ap_.tile([72, 512], BF16, tag="betaTb", name="betaTb")],
                    1: [ap_.tile([128, 512], BF16, tag="alphaTa", name="alphaTa"),
                        ap_.tile([72, 512], BF16, tag="alphaTb", name="alphaTb")]}
            for bl in range(2):        # local example index
                b = 2 * g + bl         # global example in core
                ecol = bl * 256        # column offset of example in group tiles
                zr = ap_.tile([128, 4], F32, tag="zr", name="zr")
                rz = ap_.tile([128, 4], F32, tag="rz", name="rz")
                E1 = [ap_.tile([128, 256], BF16, tag=f"E1_{i}", name=f"E1_{i}") for i in range(2)]
                ET = [ap_.tile([128, 256], BF16, tag=f"ET_{j}", name=f"ET_{j}") for j in range(2)]
                for i in range(2):     # sim rows i-chunk
                    ps = mmp.tile([128, 512], F32, tag="mm", name="mm")
                    for ki in range(2):
                        kk = MCH[ki]
                        nc.tensor.matmul(
                            out=ps[:, :256],
                            lhsT=f_T[0][ki][:kk[1] - kk[0], ecol + i * 128:ecol + (i + 1) * 128],
                            rhs=f_T[1][ki][:kk[1] - kk[0], ecol:ecol + 256],
                            start=(ki == 0), stop=(ki == 1))
                    nc.scalar.activation(
                        out=E1[i][:], in_=ps[:, :256], func=AF.Exp,
                        scale=mask_tok[0][:, 2 * b + i:2 * b + i + 1],
                        accum_out=zr[:, i:i + 1])
                for j in range(2):     # simT rows j-chunk
                    ps = mmp.tile([128, 512], F32, tag="mm", name="mm")
                    for ki in range(2):
                        kk = MCH[ki]
                        nc.tensor.matmul(
                            out=ps[:, :256],
                            lhsT=f_T[1][ki][:kk[1] - kk[0], ecol + j * 128:ecol + (j + 1) * 128],
                            rhs=f_T[0][ki][:kk[1] - kk[0], ecol:ecol + 256],
                            start=(ki == 0), stop=(ki == 1))
                    nc.scalar.activation(
                        out=ET[j][:], in_=ps[:, :256], func=AF.Exp,
                        scale=mask_tok[1][:, 2 * b + j:2 * b + j + 1],
                        accum_out=zr[:, 2 + j:3 + j])
                nc.vector.reciprocal(out=rz[:], in_=zr[:])

                # attention outputs, token-major, normalized at evacuation
                for kind in range(2):   # 0: beta (rows i), 1: alpha (rows j)
                    EWT = ET if kind == 0 else E1
                    vals = xtok[1] if kind == 0 else xtok[0]
                    for i in range(2):
                        ps = mmp.tile([128, 512], F32, tag="mm", name="mm")
                        for jj in range(2):
                            nc.tensor.matmul(
                                out=ps[:, :D],
                                lhsT=EWT[jj][:, i * 128:(i + 1) * 128],
                                rhs=vals[:, 2 * bl + jj, :],
                                start=(jj == 0), stop=(jj == 1))
                        att_tok = ap_.tile([128, D], BF16, tag=f"att{kind}", name=f"att{kind}")
                        nc.scalar.activation(
                            out=att_tok[:], in_=ps[:, :D], func=AF.Copy,
                            scale=rz[:, 2 * kind + i:2 * kind + i + 1])
                        # transpose to feature-major
                        tr = trp.tile([128, 3, 256], BF16, tag="tr", name="tr")
                        for mi, (m0, m1) in enumerate(MCH):
                            nc.tensor.transpose(
                                out=tr[:m1 - m0, mi, :128],
                                in_=att_tok[:, m0:m1], identity=identb[:])
                        for mi, (m0, m1) in enumerate(MCH):
                            nc.vector.tensor_copy(
                                out=attT[kind][mi][:m1 - m0,
                                                   ecol + i * 128:ecol + (i + 1) * 128],
                                in_=tr[:m1 - m0, mi, :128])
                if debug_taps and b == 0:
                    t = tap("E1_0", [128, 256], BF16)
                    nc.sync.dma_start(out=t[:], in_=E1[0][:])
                    t = tap("ET_0", [128, 256], BF16)
                    nc.sync.dma_start(out=t[:], in_=ET[0][:])
                    t = tap("zr0", [128, 4])
                    nc.sync.dma_start(out=t[:], in_=zr[:])
                    t = tap("attT0", [128, 512], BF16)
                    nc.sync.dma_start(out=t[:], in_=attT[0][0][:])

            # ---- G MLP per side ----
            for s in range(2):
                att = attT[0] if s == 0 else attT[1]
                hta = gp.tile([128, 512], BF16, tag="hta", name="hta")
                htb = gp.tile([73, 512], BF16, tag="htb", name="htb")
                nc.vector.memset(htb[:, :], 1.0)
                rhs_k = [xpT[s][0], xpT[s][1], att[0], att[1]]
                for mi, (m0, m1) in enumerate(MCH):
                    ps = mmp.tile([128, 512], F32, tag="mm", name="mm")
                    for ki in range(4):
                        ksz = 128 if ki % 2 == 0 else 72
                        nc.tensor.matmul(
                            out=ps[:m1 - m0, :], lhsT=gw1_k[ki][:ksz, m0:m1],
                            rhs=rhs_k[ki][:ksz, :], start=(ki == 0), stop=(ki == 3))
                    if mi == 0:
                        nc.scalar.activation(out=hta[:, :], in_=ps[:128, :],
                                             func=AF.Relu, bias=gb1_t[0][:])
                    else:
                        nc.scalar.activation(out=htb[:72, :], in_=ps[:72, :],
                                             func=AF.Relu, bias=gb1_t[1][:])
                # G2 token-major + relu + mask + v-aggregation
                for blk in range(4):
                    c = c0 + blk
                    ps = mmp.tile([128, 512], F32, tag="mm", name="mm")
                    nc.tensor.matmul(out=ps[:, :D], lhsT=hta[:, blk * 128:(blk + 1) * 128],
                                     rhs=gw2_k[0][:, :], start=True, stop=False)
                    nc.tensor.matmul(out=ps[:, :D], lhsT=htb[:, blk * 128:(blk + 1) * 128],
                                     rhs=gw2_k[1][:, :], start=False, stop=True)
                    g2o = gp.tile([128, D], BF16, tag="g2o", name="g2o")
                    nc.vector.tensor_scalar(
                        out=g2o[:], in0=ps[:, :D], scalar1=0.0,
                        scalar2=mask_tok[s][:, c:c + 1],
                        op0=OP.max, op1=OP.mult)
                    bex = c // 2
                    nc.tensor.matmul(
                        out=v_ps[s][:, :],
                        lhsT=qsel[:, BPC - 1 - bex:2 * BPC - 1 - bex],
                        rhs=g2o[:, :], start=(c == 0), stop=(c == NB - 1))
                    if debug_taps and c == 0 and s == 0:
                        t = tap("g2o0", [128, D])
                        if t is not None:
                            g2f = sp.tile([128, D], F32, tag="g2f", name="g2f")
                            nc.vector.tensor_copy(out=g2f[:], in_=g2o[:])
                            nc.sync.dma_start(out=t[:], in_=g2f[:])

        # ---------- H MLP tail ----------
        Vb = [vp.tile([BPC, D], BF16, tag=f"V{s}", name=f"V{s}") for s in range(2)]
        for s in range(2):
            nc.vector.tensor_copy(out=Vb[s][:], in_=v_ps[s][:, :])
        if debug_taps:
            t = tap("V0", [BPC, D])
            if t is not None:
                v0f = sp.tile([BPC, D], F32, tag="v0f", name="v0f")
                nc.vector.tensor_copy(out=v0f[:], in_=v_ps[0][:, :])
                nc.sync.dma_start(out=t[:], in_=v0f[:])
        # transpose V -> feature-major vT chunks
        vT = []  # 4 chunks: v1a[128,16] v1b[72,16] v2a[128,16] v2b[73,16(+ones)]
        for s in range(2):
            tr = trp.tile([128, 3, 256], BF16, tag="tr", name="tr")
            for mi, (m0, m1) in enumerate(MCH):
                nc.tensor.transpose(out=tr[:m1 - m0, mi, :BPC],
                                    in_=Vb[s][:, m0:m1], identity=identb[:BPC, :BPC])
            va = vp.tile([128, BPC], BF16, tag=f"vTa{s}", name=f"vTa{s}")
            nc.vector.tensor_copy(out=va[:], in_=tr[:128, 0, :BPC])
            szb = 73 if s == 1 else 72
            vb = vp.tile([szb, BPC], BF16, tag=f"vTb{s}", name=f"vTb{s}")
            if s == 1:
                nc.vector.memset(vb[:, :], 1.0)
            nc.vector.tensor_copy(out=vb[:72, :], in_=tr[:72, 1, :BPC])
            vT += [va, vb]

        h1a = vp.tile([128, BPC], BF16, tag="h1a", name="h1a")
        h1b = vp.tile([73, BPC], BF16, tag="h1b", name="h1b")
        nc.vector.memset(h1b[:, :], 1.0)
        for mi, (m0, m1) in enumerate(MCH):
            ps = mmp.tile([128, 512], F32, tag="mm", name="mm")
            for ki in range(4):
                ksz = [128, 72, 128, 73][ki]
                nc.tensor.matmul(out=ps[:m1 - m0, :BPC], lhsT=hw1_k[ki][:ksz, m0:m1],
                                 rhs=vT[ki][:ksz, :], start=(ki == 0), stop=(ki == 3))
            if mi == 0:
                nc.scalar.activation(out=h1a[:, :], in_=ps[:128, :BPC], func=AF.Relu)
            else:
                nc.scalar.activation(out=h1b[:72, :], in_=ps[:72, :BPC], func=AF.Relu)
        h2a = vp.tile([128, BPC], BF16, tag="h2a", name="h2a")
        h2b = vp.tile([73, BPC], BF16, tag="h2b", name="h2b")
        nc.vector.memset(h2b[:, :], 1.0)
        for mi, (m0, m1) in enumerate(MCH):
            ps = mmp.tile([128, 512], F32, tag="mm", name="mm")
            for ki in range(2):
                ksz = [128, 73][ki]
                nc.tensor.matmul(out=ps[:m1 - m0, :BPC], lhsT=hw2_k[ki][:ksz, m0:m1],
                                 rhs=[h1a, h1b][ki][:ksz, :], start=(ki == 0), stop=(ki == 1))
            if mi == 0:
                nc.scalar.activation(out=h2a[:, :], in_=ps[:128, :BPC], func=AF.Relu)
            else:
                nc.scalar.activation(out=h2b[:72, :], in_=ps[:72, :BPC], func=AF.Relu)
        ps = mmp.tile([128, 512], F32, tag="mm", name="mm")
        for ki in range(2):
            ksz = [128, 73][ki]
            nc.tensor.matmul(out=ps[:C, :BPC], lhsT=wout_k[ki][:ksz, :],
                             rhs=[h2a, h2b][ki][:ksz, :], start=(ki == 0), stop=(ki == 1))
        y_sb = vp.tile([C, BPC], F32, tag="ysb", name="ysb")
        nc.vector.tensor_copy(out=y_sb[:], in_=ps[:C, :BPC])
        nc.sync.dma_start(out=y_out[:], in_=y_sb[:])

    nc.finalize()
    return nc, taps


def _host_prep(inputs):
    """Build per-core input maps from full inputs."""
    x1 = np.asarray(inputs['x1'], dtype=np.int32)
    x2 = np.asarray(inputs['x2'], dtype=np.int32)
    len1 = np.asarray(inputs['len1'], dtype=np.int64)
    len2 = np.asarray(inputs['len2'], dtype=np.int64)
    emb = np.ascontiguousarray(np.asarray(inputs['emb'], dtype=np.float32))

    def bf(x):
        return np.ascontiguousarray(np.asarray(x, dtype=np.float32).astype(bfloat16))

    wproj = bf(inputs['W_proj'])
    fw1, fw2 = bf(inputs['F_W1']), bf(inputs['F_W2'])
    gw1 = bf(inputs['G_W1'])
    gw2e = bf(np.vstack([np.asarray(inputs['G_W2'], np.float32),
                         np.asarray(inputs['G_b2'], np.float32)[None, :]]))
    hw1e = bf(np.vstack([np.asarray(inputs['H_W1'], np.float32),
                         np.asarray(inputs['H_b1'], np.float32)[None, :]]))
    hw2e = bf(np.vstack([np.asarray(inputs['H_W2'], np.float32),
                         np.asarray(inputs['H_b2'], np.float32)[None, :]]))
    woute = bf(np.vstack([np.asarray(inputs['W_out'], np.float32),
                          np.asarray(inputs['b_out'], np.float32)[None, :]]))
    fb1 = np.ascontiguousarray(np.asarray(inputs['F_b1'], np.float32)[:, None])
    fb2 = np.ascontiguousarray(np.asarray(inputs['F_b2'], np.float32)[:, None])
    gb1 = np.ascontiguousarray(np.asarray(inputs['G_b1'], np.float32)[:, None])

    iota = np.arange(128, dtype=np.float32)[:, None].copy()
    identb = np.eye(128, dtype=np.float32).astype(bfloat16)
    qsel = np.zeros((128, 2 * BPC - 1), dtype=np.float32)
    qsel[:, BPC - 1] = 1.0
    qsel = qsel.astype(bfloat16)

    in_maps = []
    for core in range(NCORES):
        sl = slice(core * BPC, (core + 1) * BPC)

        def idx_of(x):
            return np.ascontiguousarray(
                x[sl].reshape(-1).reshape(NB, 128).T.astype(np.int32))

        def thr_of(ln):
            c = np.arange(NB)
            t = ln[sl][c // 2] - 128.0 * (c % 2)
            return np.ascontiguousarray(t.astype(np.float32))

        in_maps.append(dict(
            emb=emb, x1i=idx_of(x1), x2i=idx_of(x2),
            thr1=thr_of(len1), thr2=thr_of(len2),
            iota=iota, identb=identb, qsel=qsel,
            wproj=wproj, fw1=fw1, fw2=fw2, gw1=gw1, gw2e=gw2e,
            hw1e=hw1e, hw2e=hw2e, woute=woute,
            fb1=fb1, fb2=fb2, gb1=gb1,
        ))
    return in_maps


def run(inputs, debug_taps=False, trace=False):
    key = debug_taps
    if key not in _cache:
        _cache[key] = _build(debug_taps)
    nc, taps = _cache[key]
    in_maps = _host_prep(inputs)
    from concourse.bass_utils import run_bass_kernel_spmd
    res = run_bass_kernel_spmd(nc, in_maps, list(range(NCORES)), trace=trace)
    y = np.concatenate([r['y'].T for r in res.results], axis=0)
    return y.astype(np.float32), res


def kernel(**inputs) -> np.ndarray:
    y, _ = run(inputs)
    return y


# revision 11
# speedup vs baseline: 1.1096x; 1.1096x over previous
"""Trainium2 Bass kernel for nn_Decomposeable (decomposable attention).

Sharding: data-parallel over batch — 8 cores x 16 examples.
Layout strategy per core (T = 4096 tokens per side, blocks of 128 tokens,
token t = block*128 + p):
  - gather emb rows (bf16 cast in DMA), rowwise rsqrt-norm via exp(-0.5*ln(ss))
  - PE-transpose to feature-major eT [300, T]; project to x^T [200, T] (bf16)
  - PE-transpose x^T -> token-major x_tok blocks (attention values)
  - F-MLP feature-major; sim + simT per example on PE; ACT exp with
    per-partition mask scale + accumulated row sums (softmax without
    normalizing the matrix: fold 1/Z into the attention-output evacuation)
  - G layer1 feature-major (concat via K-chunks), layer2 token-major with
    bias via ones-row; relu+mask+evac fused; masked sum via indicator matmul
  - H-MLP + output head on [16, 400] (bias via ones-rows)
All matmul inputs bf16, fp32 PSUM accumulation.
"""
import sys
import numpy as np

for _p in ('/opt/trn_rl_repo', '/root/.axon_site'):
    if _p not in sys.path:
        sys.path.insert(0, _p)

import ml_dtypes

bfloat16 = ml_dtypes.bfloat16

B, S, V, E, D, C = 128, 256, 50000, 300, 200, 3
NCORES = 8
BPC = B // NCORES          # 16 examples per core
T = BPC * S                # 4096 tokens per side per core
NB = T // 128              # 32 blocks
NGRP = NB // 4             # 8 groups (512 tokens)

_cache = {}


def _pin_act_table_set(bacc_mod, hw_specs):
    """Make every ACT function resolve to `natural_log_exp_and_others` so the
    kernel does exactly one ACT_TABLE_LOAD (we only use exp/ln/relu/copy)."""
    import functools
    orig = hw_specs.get_activation_tables.__wrapped__

    @functools.cache
    def pinned(arch):
        t = orig(arch)
        keep = "natural_log_exp_and_others"
        if keep not in t:
            return t
        return {name: (fns if name == keep else set())
                for name, fns in t.items()}

    bacc_mod.get_activation_tables = pinned


def _build(debug_taps=False):
    from concourse import bass, bacc, mybir, tile
    from concourse import hw_specs
    _pin_act_table_set(bacc, hw_specs)

    F32 = mybir.dt.float32
    BF16 = mybir.dt.bfloat16
    I32 = mybir.dt.int32
    AF = mybir.ActivationFunctionType
    OP = mybir.AluOpType
    X = mybir.AxisListType.X

    nc = bacc.Bacc(None, num_swdge_queues=4)

    # ---- dram I/O ----
    emb = nc.dram_tensor("emb", [V, E], F32, kind="ExternalInput")
    xi = [nc.dram_tensor(f"x{s}i", [128, NB], I32, kind="ExternalInput") for s in (1, 2)]
    thr = [nc.dram_tensor(f"thr{s}", [NB], F32, kind="ExternalInput") for s in (1, 2)]
    iota_in = nc.dram_tensor("iota", [128, 1], F32, kind="ExternalInput")
    identb_in = nc.dram_tensor("identb", [128, 128], BF16, kind="ExternalInput")
    qsel_in = nc.dram_tensor("qsel", [128, 2 * BPC - 1], BF16, kind="ExternalInput")
    wproj_in = nc.dram_tensor("wproj", [E, D], BF16, kind="ExternalInput")
    fw1_in = nc.dram_tensor("fw1", [D, D], BF16, kind="ExternalInput")
    fw2_in = nc.dram_tensor("fw2", [D, D], BF16, kind="ExternalInput")
    gw1_in = nc.dram_tensor("gw1", [2 * D, D], BF16, kind="ExternalInput")
    gw2e_in = nc.dram_tensor("gw2e", [D + 1, D], BF16, kind="ExternalInput")
    hw1e_in = nc.dram_tensor("hw1e", [2 * D + 1, D], BF16, kind="ExternalInput")
    hw2e_in = nc.dram_tensor("hw2e", [D + 1, D], BF16, kind="ExternalInput")
    woute_in = nc.dram_tensor("woute", [D + 1, C], BF16, kind="ExternalInput")
    fb1_in = nc.dram_tensor("fb1", [D, 1], F32, kind="ExternalInput")
    fb2_in = nc.dram_tensor("fb2", [D, 1], F32, kind="ExternalInput")
    gb1_in = nc.dram_tensor("gb1", [D, 1], F32, kind="ExternalInput")
    y_out = nc.dram_tensor("y", [C, BPC], F32, kind="ExternalOutput")

    taps = {}

    def tap(name, shape, dtype=F32):
        if debug_taps:
            taps[name] = nc.dram_tensor(f"tap_{name}", shape, dtype, kind="ExternalOutput")
            return taps[name]
        return None

    from concourse.tile import TileContext, add_dep_helper

    from contextlib import ExitStack
    with TileContext(nc) as tc, ExitStack() as stk:
        wp = stk.enter_context(tc.tile_pool(name="wp", bufs=1))
        sp = stk.enter_context(tc.tile_pool(name="sp", bufs=1))
        ep = stk.enter_context(tc.tile_pool(name="ep", bufs=8))
        etp = stk.enter_context(tc.tile_pool(name="etp", bufs=2))
        xp = stk.enter_context(tc.tile_pool(name="xp", bufs=2))
        fp = stk.enter_context(tc.tile_pool(name="fp", bufs=2))
        ap_ = stk.enter_context(tc.tile_pool(name="ap", bufs=2))
        gp = stk.enter_context(tc.tile_pool(name="gp", bufs=3))
        vp = stk.enter_context(tc.tile_pool(name="vp", bufs=1))
        trp = stk.enter_context(tc.tile_pool(name="trp", bufs=2, space="PSUM"))
        mmp = stk.enter_context(tc.tile_pool(name="mmp", bufs=4, space="PSUM"))
        vpp = stk.enter_context(tc.tile_pool(name="vpp", bufs=1, space="PSUM"))

        # ---------- setup ----------
        idx_t = [wp.tile([128, NB], I32, tag=f"idx{s}", name=f"idx{s}") for s in range(2)]
        for s in range(2):
            nc.sync.dma_start(out=idx_t[s][:], in_=xi[s][:])
        iota_t = wp.tile([128, 1], F32, tag="iota", name="iota")
        nc.sync.dma_start(out=iota_t[:], in_=iota_in[:])
        identb = wp.tile([128, 128], BF16, tag="identb", name="identb")
        nc.sync.dma_start(out=identb[:], in_=identb_in[:])
        qsel = wp.tile([128, 2 * BPC - 1], BF16, tag="qsel", name="qsel")
        nc.sync.dma_start(out=qsel[:], in_=qsel_in[:])

        # thresholds replicated across partitions, then mask = (iota < thr)
        mask_tok = []
        for s in range(2):
            thr_rep = sp.tile([128, NB], F32, tag=f"thrrep{s}", name=f"thrrep{s}")
            nc.gpsimd.dma_start(
                out=thr_rep[:],
                in_=bass.AP(tensor=thr[s], offset=0, ap=[[0, 128], [1, NB]]))
            m = wp.tile([128, NB], F32, tag=f"mask{s}", name=f"mask{s}")
            nc.vector.tensor_tensor(
                out=m[:], in0=iota_t[:].to_broadcast([128, NB]), in1=thr_rep[:],
                op=OP.is_lt)
            mask_tok.append(m)

        # weights (bf16, chunked by K)
        def load_rows(dram, r0, r1, cols, tag):
            t = wp.tile([r1 - r0, cols], BF16, tag=tag, name=tag)
            nc.sync.dma_start(out=t[:], in_=dram[r0:r1, :])
            return t

        wproj_k = [load_rows(wproj_in, k * 128, min((k + 1) * 128, E), D, f"wp{k}")
                   for k in range(3)]
        fw1_k = [load_rows(fw1_in, 0, 128, D, "fw1a"), load_rows(fw1_in, 128, D, D, "fw1b")]
        fw2_k = [load_rows(fw2_in, 0, 128, D, "fw2a"), load_rows(fw2_in, 128, D, D, "fw2b")]
        gw1_k = [load_rows(gw1_in, 0, 128, D, "gw1a"), load_rows(gw1_in, 128, 200, D, "gw1b"),
                 load_rows(gw1_in, 200, 328, D, "gw1c"), load_rows(gw1_in, 328, 400, D, "gw1d")]
        gw2_k = [load_rows(gw2e_in, 0, 128, D, "gw2a"), load_rows(gw2e_in, 128, 201, D, "gw2b")]
        hw1_k = [load_rows(hw1e_in, 0, 128, D, "hw1a"), load_rows(hw1e_in, 128, 200, D, "hw1b"),
                 load_rows(hw1e_in, 200, 328, D, "hw1c"), load_rows(hw1e_in, 328, 401, D, "hw1d")]
        hw2_k = [load_rows(hw2e_in, 0, 128, D, "hw2a"), load_rows(hw2e_in, 128, 201, D, "hw2b")]
        wout_k = [load_rows(woute_in, 0, 128, C, "woa"), load_rows(woute_in, 128, 201, C, "wob")]

        fb1_t = [wp.tile([128, 1], F32, tag="fb1a", name="fb1a"), wp.tile([72, 1], F32, tag="fb1b", name="fb1b")]
        fb2_t = [wp.tile([128, 1], F32, tag="fb2a", name="fb2a"), wp.tile([72, 1], F32, tag="fb2b", name="fb2b")]
        gb1_t = [wp.tile([128, 1], F32, tag="gb1a", name="gb1a"), wp.tile([72, 1], F32, tag="gb1b", name="gb1b")]
        for (dr, ts_) in ((fb1_in, fb1_t), (fb2_in, fb2_t), (gb1_in, gb1_t)):
            nc.sync.dma_start(out=ts_[0][:], in_=dr[0:128, :])
            nc.sync.dma_start(out=ts_[1][:], in_=dr[128:200, :])

        # per-side norm state
        ss_t = [sp.tile([128, NB], F32, tag=f"ss{s}", name=f"ss{s}") for s in range(2)]
        rs_t = [sp.tile([128, NB], F32, tag=f"rs{s}", name=f"rs{s}") for s in range(2)]
        ln_t = [sp.tile([128, NB], F32, tag=f"ln{s}", name=f"ln{s}") for s in range(2)]

        # v accumulation psums (live whole kernel)
        v_ps = [vpp.tile([BPC, D], F32, tag=f"vps{s}", name=f"vps{s}") for s in range(2)]

        MCH = ((0, 128), (128, 200))  # feature M/K chunks of D=200

        gather_n = 0
        e_hist = []  # (tile, last_consumer_instruction) per gather, for WAR deps
        EBUFS = 8

        def gather_block(s, c):
            nonlocal gather_n
            e = ep.tile([128, E], BF16, tag="e", name="e")
            g = nc.gpsimd.indirect_dma_start(
                out=e[:], out_offset=None, in_=emb[:],
                in_offset=bass.IndirectOffsetOnAxis(ap=idx_t[s][:, c:c + 1], axis=0))
            qn = gather_n % 4
            if qn:
                g.ins.queue = f"qPoolDynamic{qn}"
            if len(e_hist) >= EBUFS:
                prev = e_hist[len(e_hist) - EBUFS][1]
                if prev is not None:
                    add_dep_helper(g.ins, prev.ins, True, "gather WAR on recycled e slot")
            e_hist.append([e, None])
            gather_n += 1
            return e, len(e_hist) - 1

        # ---------- main loop ----------
        f_T = {}   # (s) -> (fa, fb) group tiles, recreated per group
        for g in range(NGRP):
            c0 = g * 4
            xtok = {}
            xpT = {}
            for s in range(2):
                # gather + sumsq for 4 blocks
                eb = []
                for c in range(c0, c0 + 4):
                    e, hidx = gather_block(s, c)
                    sq = ep.tile([128, E], F32, tag="sq", name="sq")
                    st = nc.vector.scalar_tensor_tensor(
                        out=sq[:], in0=e[:], scalar=1.0, in1=e[:],
                        op0=OP.mult, op1=OP.mult, accum_out=ss_t[s][:, c:c + 1])
                    eb.append((e, hidx, c))
                # rsqrt wave: rs = exp(-0.5 * ln(ss))
                nc.scalar.activation(out=ln_t[s][:, c0:c0 + 4], in_=ss_t[s][:, c0:c0 + 4],
                                     func=AF.Ln)
                nc.scalar.activation(out=rs_t[s][:, c0:c0 + 4], in_=ln_t[s][:, c0:c0 + 4],
                                     func=AF.Exp, scale=-0.5)
                # scale + transpose to eT
                eT = [etp.tile([128, 512], BF16, tag=f"eT{s}{k}", name=f"eT{s}{k}") for k in range(3)]
                for wv in range(2):  # waves of 2 blocks
                    tr = trp.tile([128, 3, 256], BF16, tag="tr", name="tr")
                    for half in range(2):
                        e, hidx, c = eb[wv * 2 + half]
                        ebf = ep.tile([128, E], BF16, tag="ebf", name="ebf")
                        sc = nc.vector.tensor_scalar(
                            out=ebf[:], in0=e[:], scalar1=rs_t[s][:, c:c + 1],
                            scalar2=None, op0=OP.mult)
                        e_hist[hidx][1] = sc
                        for k in range(3):
                            ksz = 128 if k < 2 else E - 256
                            nc.tensor.transpose(
                                out=tr[:ksz, k, half * 128:(half + 1) * 128],
                                in_=ebf[:, k * 128:k * 128 + ksz],
                                identity=identb[:])
                    for k in range(3):
                        ksz = 128 if k < 2 else E - 256
                        nc.vector.tensor_copy(
                            out=eT[k][:ksz, wv * 256:(wv + 1) * 256],
                            in_=tr[:ksz, k, :])
                # projection -> xpT (feature-major x^T, bf16)
                xpT[s] = [xp.tile([128, 512], BF16, tag=f"xpT{s}0", name=f"xpTa{s}"),
                          xp.tile([72, 512], BF16, tag=f"xpT{s}1", name=f"xpTb{s}")]
                for mi, (m0, m1) in enumerate(MCH):
                    ps = mmp.tile([128, 512], F32, tag="mm", name="mm")
                    for k in range(3):
                        ksz = 128 if k < 2 else E - 256
                        nc.tensor.matmul(
                            out=ps[:m1 - m0, :], lhsT=wproj_k[k][:ksz, m0:m1],
                            rhs=eT[k][:ksz, :], start=(k == 0), stop=(k == 2))
                    nc.scalar.copy(out=xpT[s][mi][:, :], in_=ps[:m1 - m0, :])
                # token-major x blocks (for attention values)
                xtok[s] = xp.tile([128, 4, D], BF16, tag=f"xtok{s}", name=f"xtok{s}")
                for blk in range(4):
                    tr = trp.tile([128, 3, 256], BF16, tag="tr", name="tr")
                    for mi, (m0, m1) in enumerate(MCH):
                        nc.tensor.transpose(
                            out=tr[:128, 0, m0:m1],
                            in_=xpT[s][mi][:m1 - m0, blk * 128:(blk + 1) * 128],
                            identity=identb[:m1 - m0, :m1 - m0])
                    nc.vector.tensor_copy(out=xtok[s][:, blk, :], in_=tr[:, 0, 0:D])
                # F MLP (feature-major)
                fh = [fp.tile([128, 512], BF16, tag=f"fh{s}0", name=f"fha{s}"),
                      fp.tile([72, 512], BF16, tag=f"fh{s}1", name=f"fhb{s}")]
                for mi, (m0, m1) in enumerate(MCH):
                    ps = mmp.tile([128, 512], F32, tag="mm", name="mm")
                    for ki, (k0, k1) in enumerate(MCH):
                        nc.tensor.matmul(
                            out=ps[:m1 - m0, :], lhsT=fw1_k[ki][:k1 - k0, m0:m1],
                            rhs=xpT[s][ki][:k1 - k0, :], start=(ki == 0), stop=(ki == 1))
                    nc.scalar.activation(out=fh[mi][:, :], in_=ps[:m1 - m0, :],
                                         func=AF.Relu, bias=fb1_t[mi][:])
                fT = [fp.tile([128, 512], BF16, tag=f"fT{s}0", name=f"fTa{s}"),
                      fp.tile([72, 512], BF16, tag=f"fT{s}1", name=f"fTb{s}")]
                for mi, (m0, m1) in enumerate(MCH):
                    ps = mmp.tile([128, 512], F32, tag="mm", name="mm")
                    for ki, (k0, k1) in enumerate(MCH):
                        nc.tensor.matmul(
                            out=ps[:m1 - m0, :], lhsT=fw2_k[ki][:k1 - k0, m0:m1],
                            rhs=fh[ki][:k1 - k0, :], start=(ki == 0), stop=(ki == 1))
                    nc.scalar.activation(out=fT[mi][:, :], in_=ps[:m1 - m0, :],
                                         func=AF.Relu, bias=fb2_t[mi][:])
                f_T[s] = fT

                if debug_taps and g == 0 and s == 0:
                    for nm, src in (("eT0", eT[0]), ("xpT0", xpT[0][0]),
                                    ("fT0", fT[0])):
                        t = tap(nm, [128, 512], BF16)
                        nc.sync.dma_start(out=t[:], in_=src[:])
                    t = tap("xtok0", [128, 4 * D], BF16)
                    nc.sync.dma_start(out=t[:], in_=xtok[0][:].rearrange("p a b -> p (a b)"))

            # ---- attention + G for the 2 examples of this group ----
            # group-level betaT/alphaT (feature-major attention outputs)
            attT = {0: [ap_.tile([128, 512], BF16, tag="betaTa", name="betaTa"),
                        ap_.tile([72, 512], BF16, tag="betaTb", name="betaTb")],
                    1: [ap_.tile([128, 512], BF16, tag="alphaTa", name="alphaTa"),
                        ap_.tile([72, 512], BF16, tag="alphaTb", name="alphaTb")]}
            for bl in range(2):        # local example index
                b = 2 * g + bl         # global example in core
                ecol = bl * 256        # column offset of example in group tiles
                zr = ap_.tile([128, 4], F32, tag="zr", name="zr")
                rz = ap_.tile([128, 4], F32, tag="rz", name="rz")
                E1 = [ap_.tile([128, 256], BF16, tag=f"E1_{i}", name=f"E1_{i}") for i in range(2)]
                ET = [ap_.tile([128, 256], BF16, tag=f"ET_{j}", name=f"ET_{j}") for j in range(2)]
                for i in range(2):     # sim rows i-chunk
                    ps = mmp.tile([128, 512], F32, tag="mm", name="mm")
                    for ki in range(2):
                        kk = MCH[ki]
                        nc.tensor.matmul(
                            out=ps[:, :256],
                            lhsT=f_T[0][ki][:kk[1] - kk[0], ecol + i * 128:ecol + (i + 1) * 128],
                            rhs=f_T[1][ki][:kk[1] - kk[0], ecol:ecol + 256],
                            start=(ki == 0), stop=(ki == 1))
                    nc.scalar.activation(
                        out=E1[i][:], in_=ps[:, :256], func=AF.Exp,
                        scale=mask_tok[0][:, 2 * b + i:2 * b + i + 1],
                        accum_out=zr[:, i:i + 1])
                for j in range(2):     # simT rows j-chunk
                    ps = mmp.tile([128, 512], F32, tag="mm", name="mm")
                    for ki in range(2):
                        kk = MCH[ki]
                        nc.tensor.matmul(
                            out=ps[:, :256],
                            lhsT=f_T[1][ki][:kk[1] - kk[0], ecol + j * 128:ecol + (j + 1) * 128],
                            rhs=f_T[0][ki][:kk[1] - kk[0], ecol:ecol + 256],
                            start=(ki == 0), stop=(ki == 1))
                    nc.scalar.activation(
                        out=ET[j][:], in_=ps[:, :256], func=AF.Exp,
                        scale=mask_tok[1][:, 2 * b + j:2 * b + j + 1],
                        accum_out=zr[:, 2 + j:3 + j])
                nc.vector.reciprocal(out=rz[:], in_=zr[:])

                # attention outputs, token-major, normalized at evacuation
                for kind in range(2):   # 0: beta (rows i), 1: alpha (rows j)
                    EWT = ET if kind == 0 else E1
                    vals = xtok[1] if kind == 0 else xtok[0]
                    for i in range(2):
                        ps = mmp.tile([128, 512], F32, tag="mm", name="mm")
                        for jj in range(2):
                            nc.tensor.matmul(
                                out=ps[:, :D],
                                lhsT=EWT[jj][:, i * 128:(i + 1) * 128],
                                rhs=vals[:, 2 * bl + jj, :],
                                start=(jj == 0), stop=(jj == 1))
                        att_tok = ap_.tile([128, D], BF16, tag=f"att{kind}", name=f"att{kind}")
                        nc.scalar.activation(
                            out=att_tok[:], in_=ps[:, :D], func=AF.Copy,
                            scale=rz[:, 2 * kind + i:2 * kind + i + 1])
                        # transpose to feature-major
                        tr = trp.tile([128, 3, 256], BF16, tag="tr", name="tr")
                        for mi, (m0, m1) in enumerate(MCH):
                            nc.tensor.transpose(
                                out=tr[:m1 - m0, mi, :128],
                                in_=att_tok[:, m0:m1], identity=identb[:])
                        for mi, (m0, m1) in enumerate(MCH):
                            nc.vector.tensor_copy(
                                out=attT[kind][mi][:m1 - m0,
                                                   ecol + i * 128:ecol + (i + 1) * 128],
                                in_=tr[:m1 - m0, mi, :128])
                if debug_taps and b == 0:
                    t = tap("E1_0", [128, 256], BF16)
                    nc.sync.dma_start(out=t[:], in_=E1[0][:])
                    t = tap("ET_0", [128, 256], BF16)
                    nc.sync.dma_start(out=t[:], in_=ET[0][:])
                    t = tap("zr0", [128, 4])
                    nc.sync.dma_start(out=t[:], in_=zr[:])
                    t = tap("attT0", [128, 512], BF16)
                    nc.sync.dma_start(out=t[:], in_=attT[0][0][:])

            # ---- G MLP per side ----
            for s in range(2):
                att = attT[0] if s == 0 else attT[1]
                hta = gp.tile([128, 512], BF16, tag="hta", name="hta")
                htb = gp.tile([73, 512], BF16, tag="htb", name="htb")
                nc.vector.memset(htb[:, :], 1.0)
                rhs_k = [xpT[s][0], xpT[s][1], att[0], att[1]]
                for mi, (m0, m1) in enumerate(MCH):
                    ps = mmp.tile([128, 512], F32, tag="mm", name="mm")
                    for ki in range(4):
                        ksz = 128 if ki % 2 == 0 else 72
                        nc.tensor.matmul(
                            out=ps[:m1 - m0, :], lhsT=gw1_k[ki][:ksz, m0:m1],
                            rhs=rhs_k[ki][:ksz, :], start=(ki == 0), stop=(ki == 3))
                    if mi == 0:
                        nc.scalar.activation(out=hta[:, :], in_=ps[:128, :],
                                             func=AF.Relu, bias=gb1_t[0][:])
                    else:
                        nc.scalar.activation(out=htb[:72, :], in_=ps[:72, :],
                                             func=AF.Relu, bias=gb1_t[1][:])
                # G2 token-major + relu + mask + v-aggregation
                for blk in range(4):
                    c = c0 + blk
                    ps = mmp.tile([128, 512], F32, tag="mm", name="mm")
                    nc.tensor.matmul(out=ps[:, :D], lhsT=hta[:, blk * 128:(blk + 1) * 128],
                                     rhs=gw2_k[0][:, :], start=True, stop=False)
                    nc.tensor.matmul(out=ps[:, :D], lhsT=htb[:, blk * 128:(blk + 1) * 128],
                                     rhs=gw2_k[1][:, :], start=False, stop=True)
                    g2o = gp.tile([128, D], BF16, tag="g2o", name="g2o")
                    nc.vector.tensor_scalar(
                        out=g2o[:], in0=ps[:, :D], scalar1=0.0,
                        scalar2=mask_tok[s][:, c:c + 1],
                        op0=OP.max, op1=OP.mult)
                    bex = c // 2
                    nc.tensor.matmul(
                        out=v_ps[s][:, :],
                        lhsT=qsel[:, BPC - 1 - bex:2 * BPC - 1 - bex],
                        rhs=g2o[:, :], start=(c == 0), stop=(c == NB - 1))
                    if debug_taps and c == 0 and s == 0:
                        t = tap("g2o0", [128, D])
                        if t is not None:
                            g2f = sp.tile([128, D], F32, tag="g2f", name="g2f")
                            nc.vector.tensor_copy(out=g2f[:], in_=g2o[:])
                            nc.sync.dma_start(out=t[:], in_=g2f[:])

        # ---------- H MLP tail ----------
        Vb = [vp.tile([BPC, D], BF16, tag=f"V{s}", name=f"V{s}") for s in range(2)]
        for s in range(2):
            nc.vector.tensor_copy(out=Vb[s][:], in_=v_ps[s][:, :])
        if debug_taps:
            t = tap("V0", [BPC, D])
            if t is not None:
                v0f = sp.tile([BPC, D], F32, tag="v0f", name="v0f")
                nc.vector.tensor_copy(out=v0f[:], in_=v_ps[0][:, :])
                nc.sync.dma_start(out=t[:], in_=v0f[:])
        # transpose V -> feature-major vT chunks
        vT = []  # 4 chunks: v1a[128,16] v1b[72,16] v2a[128,16] v2b[73,16(+ones)]
        for s in range(2):
            tr = trp.tile([128, 3, 256], BF16, tag="tr", name="tr")
            for mi, (m0, m1) in enumerate(MCH):
                nc.tensor.transpose(out=tr[:m1 - m0, mi, :BPC],
                                    in_=Vb[s][:, m0:m1], identity=identb[:BPC, :BPC])
            va = vp.tile([128, BPC], BF16, tag=f"vTa{s}", name=f"vTa{s}")
            nc.vector.tensor_copy(out=va[:], in_=tr[:128, 0, :BPC])
            szb = 73 if s == 1 else 72
            vb = vp.tile([szb, BPC], BF16, tag=f"vTb{s}", name=f"vTb{s}")
            if s == 1:
                nc.vector.memset(vb[:, :], 1.0)
            nc.vector.tensor_copy(out=vb[:72, :], in_=tr[:72, 1, :BPC])
            vT += [va, vb]

        h1a = vp.tile([128, BPC], BF16, tag="h1a", name="h1a")
        h1b = vp.tile([73, BPC], BF16, tag="h1b", name="h1b")
        nc.vector.memset(h1b[:, :], 1.0)
        for mi, (m0, m1) in enumerate(MCH):
            ps = mmp.tile([128, 512], F32, tag="mm", name="mm")
            for ki in range(4):
                ksz = [128, 72, 128, 73][ki]
                nc.tensor.matmul(out=ps[:m1 - m0, :BPC], lhsT=hw1_k[ki][:ksz, m0:m1],
                                 rhs=vT[ki][:ksz, :], start=(ki == 0), stop=(ki == 3))
            if mi == 0:
                nc.scalar.activation(out=h1a[:, :], in_=ps[:128, :BPC], func=AF.Relu)
            else:
                nc.scalar.activation(out=h1b[:72, :], in_=ps[:72, :BPC], func=AF.Relu)
        h2a = vp.tile([128, BPC], BF16, tag="h2a", name="h2a")
        h2b = vp.tile([73, BPC], BF16, tag="h2b", name="h2b")
        nc.vector.memset(h2b[:, :], 1.0)
        for mi, (m0, m1) in enumerate(MCH):
            ps = mmp.tile([128, 512], F32, tag="mm", name="mm")
            for ki in range(2):
                ksz = [128, 73][ki]
                nc.tensor.matmul(out=ps[:m1 - m0, :BPC], lhsT=hw2_k[ki][:ksz, m0:m1],
                                 rhs=[h1a, h1b][ki][:ksz, :], start=(ki == 0), stop=(ki == 1))
            if mi == 0:
                nc.scalar.activation(out=h2a[:, :], in_=ps[:128, :BPC], func=AF.Relu)
            else:
                nc.scalar.activation(out=h2b[:72, :], in_=ps[:72, :BPC], func=AF.Relu)
        ps = mmp.tile([128, 512], F32, tag="mm", name="mm")
        for ki in range(2):
            ksz = [128, 73][ki]
            nc.tensor.matmul(out=ps[:C, :BPC], lhsT=wout_k[ki][:ksz, :],
                             rhs=[h2a, h2b][ki][:ksz, :], start=(ki == 0), stop=(ki == 1))
        y_sb = vp.tile([C, BPC], F32, tag="ysb", name="ysb")
        nc.vector.tensor_copy(out=y_sb[:], in_=ps[:C, :BPC])
        nc.sync.dma_start(out=y_out[:], in_=y_sb[:])

    nc.finalize()
    return nc, taps


def _host_prep(inputs):
    """Build per-core input maps from full inputs."""
    x1 = np.asarray(inputs['x1'], dtype=np.int32)
    x2 = np.asarray(inputs['x2'], dtype=np.int32)
    len1 = np.asarray(inputs['len1'], dtype=np.int64)
    len2 = np.asarray(inputs['len2'], dtype=np.int64)
    emb = np.ascontiguousarray(np.asarray(inputs['emb'], dtype=np.float32))

    def bf(x):
        return np.ascontiguousarray(np.asarray(x, dtype=np.float32).astype(bfloat16))

    wproj = bf(inputs['W_proj'])
    fw1, fw2 = bf(inputs['F_W1']), bf(inputs['F_W2'])
    gw1 = bf(inputs['G_W1'])
    gw2e = bf(np.vstack([np.asarray(inputs['G_W2'], np.float32),
                         np.asarray(inputs['G_b2'], np.float32)[None, :]]))
    hw1e = bf(np.vstack([np.asarray(inputs['H_W1'], np.float32),
                         np.asarray(inputs['H_b1'], np.float32)[None, :]]))
    hw2e = bf(np.vstack([np.asarray(inputs['H_W2'], np.float32),
                         np.asarray(inputs['H_b2'], np.float32)[None, :]]))
    woute = bf(np.vstack([np.asarray(inputs['W_out'], np.float32),
                          np.asarray(inputs['b_out'], np.float32)[None, :]]))
    fb1 = np.ascontiguousarray(np.asarray(inputs['F_b1'], np.float32)[:, None])
    fb2 = np.ascontiguousarray(np.asarray(inputs['F_b2'], np.float32)[:, None])
    gb1 = np.ascontiguousarray(np.asarray(inputs['G_b1'], np.float32)[:, None])

    iota = np.arange(128, dtype=np.float32)[:, None].copy()
    identb = np.eye(128, dtype=np.float32).astype(bfloat16)
    qsel = np.zeros((128, 2 * BPC - 1), dtype=np.float32)
    qsel[:, BPC - 1] = 1.0
    qsel = qsel.astype(bfloat16)

    in_maps = []
    for core in range(NCORES):
        sl = slice(core * BPC, (core + 1) * BPC)

        def idx_of(x):
            return np.ascontiguousarray(
                x[sl].reshape(-1).reshape(NB, 128).T.astype(np.int32))

        def thr_of(ln):
            c = np.arange(NB)
            t = ln[sl][c // 2] - 128.0 * (c % 2)
            return np.ascontiguousarray(t.astype(np.float32))

        in_maps.append(dict(
            emb=emb, x1i=idx_of(x1), x2i=idx_of(x2),
            thr1=thr_of(len1), thr2=thr_of(len2),
            iota=iota, identb=identb, qsel=qsel,
            wproj=wproj, fw1=fw1, fw2=fw2, gw1=gw1, gw2e=gw2e,
            hw1e=hw1e, hw2e=hw2e, woute=woute,
            fb1=fb1, fb2=fb2, gb1=gb1,
        ))
    return in_maps


def run(inputs, debug_taps=False, trace=False):
    key = debug_taps
    if key not in _cache:
        _cache[key] = _build(debug_taps)
    nc, taps = _cache[key]
    in_maps = _host_prep(inputs)
    from concourse.bass_utils import run_bass_kernel_spmd
    res = run_bass_kernel_spmd(nc, in_maps, list(range(NCORES)), trace=trace)
    y = np.concatenate([r['y'].T for r in res.results], axis=0)
    return y.astype(np.float32), res


def kernel(**inputs) -> np.ndarray:
    y, _ = run(inputs)
    return y
